# revision 19
# baseline (speedup 1.0000x reference)
"""Trainium2 Bass kernel for nn_GCNNLayer_56796647522692 (GCN message-passing layer).

Math (per flattened token row j of M = BNK*L = 25600, D = O = 1024, R = 50):
    idx      = adj_arc_in[:,0]*L + adj_arc_in[:,1]          (gather source rows)
    in_      = rep_[idx] @ W_in + b_in[lab]                 (gather commutes with matmul)
    in_gate  = rep_[idx] @ W_gate_in + b_gate_in[lab]
    same_    = rep_ @ W_self
    same_g   = rep_ @ W_gate_self
    w_in     = adj_mask_in^2  * sigmoid(in_gate)
    w_self   = adj_mask_loop^2 * sigmoid(same_g)            (adj_mask_loop == 1)
    out      = relu(in_*w_in + same_*w_self) * mask

Sharding: data-parallel over rows on 8 cores. The host performs the row gather
(rep_[idx]), lhsT-layout transposes, b_in[lab] row gather, and mask packing; each
core runs a dense fused matmul kernel.

Row compaction (v2): rows with mask==0 are dropped entirely (output stays 0) and
rows whose only contribution is the self loop (adj_mask_in==0) are packed into
"self-only" tiles that skip the in-arc matmuls. Tile counts (TB both-tiles, TS
self-tiles per core) are derived from the input masks, so dense inputs fall back
to TB=25/TS=0 automatically. Self-only rows that spill into both-tiles get a
zeroed gather row + zero b row (their in-term vanishes; m2i=0 also kills it).

Device layout (token-major, out partitions = tokens): lhsT = X^T 128x128 chunks
(stationary), rhs = W 128x512 chunks (moving), PSUM accumulates over 8 k-tiles
(fp16 in, fp32 acc). Gates ride the same lhsT as N=2 matmuls. Per m-tile, ALL
matmuls are emitted before the combine ("batch" order) — emitting the n0-combine
between the n0 and n1 matmul blocks measured 250-270us vs ~160us for batch.
The k-loop is outermost within a source so the gate + both n-chunk matmuls share
one stationary load per (s,k) (the n-outer form saturates the LDWEIGHTS path on
the gate-riding n0 loop: 214ns of LDW per 214ns of stream).
Combine on ACT/DVE: sigmoid -> fused scalar_tensor_tensor chain -> relu*mask ->
DMA out (f16; host casts back to f32).

Measured per-core steady state ~145-155us vs a 150.2us f16 PE-stream floor
(gate matmuls ~17us, overlapped). fp8 DoubleRow was evaluated and rejected:
e4m3 quantization of both operands gives 3.9e-2 rel err vs the 2e-2 gate.
"""

import numpy as np
import ml_dtypes

import concourse.bass as bass
import concourse.tile as tile
from concourse import bacc, mybir
from concourse.bass_utils import run_bass_kernel_spmd

# ---- problem dims (hardcoded per contract) ----
BNK, L, D, O, R = 200, 128, 1024, 1024, 50
M = BNK * L              # 25600
NCORES = 8
P = 128
KT = D // P              # 8 k-tiles
NFREE = 512
NT = O // NFREE          # 2 n-chunks

import os
MM_MODE = os.environ.get("GCN_MM_MODE", "f16")
# bench-only: repeat the whole compute loop R times inside the NEFF so kernel
# time dominates the per-exec RPC overhead; slope between two R values gives HW time
REPEAT = int(os.environ.get("GCN_REPEAT", "1"))
# b_in[lab] add: "dve" = host-gathered rows added on VectorE; "pe" = one-hot matmul
BIAS = os.environ.get("GCN_BIAS", "dve")
# timing probe only (wrong math): skip gate matmuls to measure their PE cost
NOGATE = os.environ.get("GCN_NOGATE", "0") == "1"
# 6 big + 2 gate psum banks: with 5, the 4 allocs/tile rotating over 5 banks
# hit a recurring alignment stall (~837ns every 4th tile in the modeled
# timeline, −9us/rep confirmed on HW A/B vs 5/3)
PSUM_BIG = int(os.environ.get("GCN_PSUM_BIG", "6"))
PSUM_G = int(os.environ.get("GCN_PSUM_G", "2"))
# input-tile DMA prefetch depth: 4 leaves an ~837ns PE stall every 4th tile
# in the modeled timeline (ring turnaround vs DMA queueing); 6 removes it
XBUFS = int(os.environ.get("GCN_XBUFS", "6"))
# output dtype from device: f16 halves the out DMA; host casts back to f32
OUT16 = os.environ.get("GCN_OUT16", "1") == "1"
# probe: disable row compaction (all rows dense as both-tiles)
DENSE = os.environ.get("GCN_DENSE", "0") == "1"

_DT = {
    "bf16": (mybir.dt.bfloat16, ml_dtypes.bfloat16),
    "f16": (mybir.dt.float16, np.float16),
    "f32r": (mybir.dt.float32r, np.float32),
    "f32": (mybir.dt.float32, np.float32),
}
MM_DT, MM_NP = _DT[MM_MODE]
F32 = mybir.dt.float32
OUT_DT = mybir.dt.float16 if OUT16 else F32
OUT_NP = np.float16 if OUT16 else np.float32
AF = mybir.ActivationFunctionType


def build_bass(tb, ts):
    """tb both-tiles (2 sources) + ts self-only tiles (1 source) per core."""
    nc = bacc.Bacc("TRN2", target_bir_lowering=False, debug=False, num_devices=NCORES)

    xtb = nc.dram_tensor("xtb", (tb, P, 2, KT, P), MM_DT, kind="ExternalInput").ap()
    if ts:
        xts = nc.dram_tensor("xts", (ts, P, KT, P), MM_DT, kind="ExternalInput").ap()
    w = nc.dram_tensor("w", (2, D, O), MM_DT, kind="ExternalInput").ap()
    wg = nc.dram_tensor("wg", (D, 2), MM_DT, kind="ExternalInput").ap()
    # mask channels per both-tile: 0 m2i, 1 unused, 2 unused, 3 m2l, 4 mask, 5 bg
    msk = nc.dram_tensor("msk", (P, tb, 6), F32, kind="ExternalInput").ap()
    if ts:
        msks = nc.dram_tensor("msks", (P, ts, 1), F32, kind="ExternalInput").ap()
    if BIAS == "dve":
        brow = nc.dram_tensor("brow", (tb, P, O), MM_DT, kind="ExternalInput").ap()
    else:
        bau = nc.dram_tensor("bau", (R, O), MM_DT, kind="ExternalInput").ap()
        oht = nc.dram_tensor("oht", (R, tb * P), MM_DT, kind="ExternalInput").ap()
    out = nc.dram_tensor("out", ((tb + ts) * P, O), OUT_DT, kind="ExternalOutput").ap()

    with tile.TileContext(nc) as tc:
        with (
            tc.tile_pool(name="const", bufs=1) as const,
            tc.tile_pool(name="xtp", bufs=XBUFS) as xtp,
            tc.tile_pool(name="colp", bufs=8) as colp,
            tc.tile_pool(name="tmp", bufs=6) as tmpp,
            tc.tile_pool(name="outp", bufs=6) as outp,
            tc.tile_pool(name="psum", bufs=PSUM_BIG, space="PSUM") as psum,
            tc.tile_pool(name="psg", bufs=PSUM_G, space="PSUM") as psg,
        ):
            # first m-tile's inputs + small constants before the 4MB weight
            # preload so the first matmuls are not queued behind it
            xt0 = xtp.tile([P, 2, KT, P], MM_DT, tag="xt_t", name="xt0")
            nc.sync.dma_start(xt0[:], xtb[0])
            br0 = None
            if BIAS == "dve":
                br0 = xtp.tile([P, O], MM_DT, tag="brow", name="br0")
                nc.sync.dma_start(br0[:], brow[0])
            wg_sb = const.tile([P, KT, 2], MM_DT)
            nc.sync.dma_start(wg_sb[:], wg.rearrange("(k p) g -> p k g", p=P))
            if BIAS == "pe":
                bau_sb = const.tile([R, O], MM_DT)
                nc.sync.dma_start(bau_sb[:], bau)
                oht_sb = const.tile([R, tb * P], MM_DT)
                nc.sync.dma_start(oht_sb[:], oht)
            msk_sb = const.tile([P, tb, 6], F32)
            nc.sync.dma_start(msk_sb[:], msk)
            if ts:
                msks_sb = const.tile([P, ts, 1], F32)
                nc.sync.dma_start(msks_sb[:], msks)

            # ---- weight preload: per-(s,k) tiles, DMA'd in consumption order
            # (s-major: the first tile streams s0 k0..7 before any s1 chunk) ----
            w_t = [[const.tile([P, O], MM_DT, name=f"w_{s}_{k}") for k in range(KT)]
                   for s in range(2)]
            for s in range(2):
                for k in range(KT):
                    nc.sync.dma_start(w_t[s][k][:], w[s, k * P:(k + 1) * P, :])

            first = True
            for _ in range(REPEAT):
                # ---------- both-tiles: in + self ----------
                for m in range(tb):
                    if first:
                        xt_t, br_t, first = xt0, br0, False
                    else:
                        xt_t = xtp.tile([P, 2, KT, P], MM_DT, tag="xt_t", name="xt_t")
                        nc.sync.dma_start(xt_t[:], xtb[m])
                        if BIAS == "dve":
                            br_t = xtp.tile([P, O], MM_DT, tag="brow", name="br_t")
                            nc.sync.dma_start(br_t[:], brow[m])

                    # gate psum: col 0 = Xin@wg_in, col 3 = Xself@wg_self
                    g_ps = psg.tile([P, 4], F32)
                    wcol = colp.tile([P, 4], F32)

                    def src_block(s, m=m, xt_t=xt_t, g_ps=g_ps):
                        # k-outer so the gate + both n-chunk matmuls share one
                        # stationary load per (s,k): 3 LDW per 428ns of stream
                        # stays hidden, vs the n-outer form where the
                        # gate-riding n0 loop is LDW-saturated (214ns/214ns)
                        ps_n = [psum.tile([P, NFREE], F32, tag="big",
                                          name=f"ps{s}{n}") for n in range(NT)]
                        gsl = slice(0, 2) if s == 0 else slice(2, 4)
                        for k in range(KT):
                            lhsT = xt_t[:, s, k]
                            last = k == KT - 1
                            # big n0 first so the tile can start before a gate
                            # psum bank frees; gate rides the same LDW after it
                            nc.tensor.matmul(
                                ps_n[0][:], lhsT, w_t[s][k][:, 0:NFREE],
                                start=(k == 0),
                                stop=(last and (s == 1 or BIAS == "dve")))
                            if not NOGATE:
                                nc.tensor.matmul(
                                    g_ps[:, gsl], lhsT, wg_sb[:, k, 0:2],
                                    start=(k == 0), stop=last)
                            for n in range(1, NT):
                                nc.tensor.matmul(
                                    ps_n[n][:], lhsT,
                                    w_t[s][k][:, n * NFREE:(n + 1) * NFREE],
                                    start=(k == 0),
                                    stop=(last and (s == 1 or BIAS == "dve")))
                        if BIAS == "pe" and s == 0:
                            for n in range(NT):
                                nc.tensor.matmul(
                                    ps_n[n][:], oht_sb[:, m * P:(m + 1) * P],
                                    bau_sb[:, n * NFREE:(n + 1) * NFREE],
                                    start=False, stop=True)
                        return ps_n

                    i_ps = src_block(0)
                    s_ps = src_block(1)

                    # w = mask_soft^2 * sigmoid(gate + gate_bias); cols 0, 3 valid
                    if NOGATE:
                        nc.vector.tensor_copy(wcol[:], msk_sb[:, m, 0:4])
                    else:
                        nc.scalar.activation(wcol[:, 0:2], g_ps[:, 0:2], AF.Sigmoid,
                                             bias=msk_sb[:, m, 5:6])
                        nc.scalar.activation(wcol[:, 2:4], g_ps[:, 2:4], AF.Sigmoid)
                        nc.vector.tensor_tensor(wcol[:], wcol[:], msk_sb[:, m, 0:4],
                                                mybir.AluOpType.mult)

                    for n in range(NT):
                        # out = relu((I+b)*w_in + S*w_self) * mask
                        #     = relu(I*w_in + (S*w_self + b*w_in)) * mask
                        t1 = tmpp.tile([P, NFREE], F32, tag="t1", name="t1")
                        t2 = tmpp.tile([P, NFREE], F32, tag="t2", name="t2")
                        if BIAS == "dve":
                            t3 = tmpp.tile([P, NFREE], F32, tag="t3", name="t3")
                            nc.scalar.mul(t3[:], br_t[:, n * NFREE:(n + 1) * NFREE],
                                          wcol[:, 0:1])
                            nc.vector.scalar_tensor_tensor(
                                t2[:], s_ps[n][:], wcol[:, 3:4], t3[:],
                                mybir.AluOpType.mult, mybir.AluOpType.add)
                        else:
                            nc.vector.tensor_scalar_mul(t2[:], s_ps[n][:],
                                                        wcol[:, 3:4])
                        nc.vector.scalar_tensor_tensor(
                            t1[:], i_ps[n][:], wcol[:, 0:1], t2[:],
                            mybir.AluOpType.mult, mybir.AluOpType.add)
                        o_t = outp.tile([P, NFREE], OUT_DT, tag="ot", name="o_t")
                        nc.scalar.activation(o_t[:], t1[:], AF.Relu,
                                             scale=msk_sb[:, m, 4:5])
                        nc.sync.dma_start(
                            out[m * P:(m + 1) * P, n * NFREE:(n + 1) * NFREE], o_t[:])

                # ---------- self-only tiles ----------
                for m in range(ts):
                    xt_t = xtp.tile([P, KT, P], MM_DT, tag="xts_t", name="xts_t")
                    nc.sync.dma_start(xt_t[:], xts[m])
                    g_ps = psg.tile([P, 2], F32)
                    s_ps = [psum.tile([P, NFREE], F32, tag="big", name=f"pss{n}")
                            for n in range(NT)]
                    for k in range(KT):
                        lhsT = xt_t[:, k]
                        nc.tensor.matmul(
                            s_ps[0][:], lhsT, w_t[1][k][:, 0:NFREE],
                            start=(k == 0), stop=(k == KT - 1))
                        if not NOGATE:
                            nc.tensor.matmul(g_ps[:], lhsT, wg_sb[:, k, 0:2],
                                             start=(k == 0), stop=(k == KT - 1))
                        for n in range(1, NT):
                            nc.tensor.matmul(
                                s_ps[n][:], lhsT,
                                w_t[1][k][:, n * NFREE:(n + 1) * NFREE],
                                start=(k == 0), stop=(k == KT - 1))
                    wcol = colp.tile([P, 2], F32, tag="wcols", name="wcols")
                    if NOGATE:
                        nc.vector.tensor_copy(wcol[:, 1:2], msks_sb[:, m, 0:1])
                    else:
                        nc.scalar.activation(wcol[:], g_ps[:], AF.Sigmoid)
                        # fold mask into the self weight (col 1 = sigma(g_self))
                        nc.vector.tensor_tensor(wcol[:, 1:2], wcol[:, 1:2],
                                                msks_sb[:, m, 0:1],
                                                mybir.AluOpType.mult)
                    for n in range(NT):
                        o_t = outp.tile([P, NFREE], OUT_DT, tag="ot", name="o_ts")
                        nc.scalar.activation(o_t[:], s_ps[n][:], AF.Relu,
                                             scale=wcol[:, 1:2])
                        nc.sync.dma_start(
                            out[(tb + m) * P:(tb + m + 1) * P,
                                n * NFREE:(n + 1) * NFREE], o_t[:])

    nc.compile()
    return nc


_NC = {}


def _get_nc(tb, ts):
    key = (tb, ts, REPEAT, MM_MODE, BIAS, NOGATE, PSUM_BIG, PSUM_G, OUT16)
    if key not in _NC:
        _NC.clear()
        _NC[key] = build_bass(tb, ts)
    return _NC[key]


def plan_rows(mask_flat, ami):
    """Assign rows to cores/slots. Returns (tb, ts, per-core row-index arrays
    rows_b, rows_s, and boolean borrow flags for both-slots)."""
    active = mask_flat != 0.0
    both_idx = np.where(active & (ami != 0.0))[0]
    self_idx = np.where(active & (ami == 0.0))[0]
    bsplit = np.array_split(both_idx, NCORES)
    ssplit = np.array_split(self_idx, NCORES)
    nb_max = max(len(b) for b in bsplit)
    tb = max(1, -(-nb_max // P))
    rows_b, rows_s, borrow_n = [], [], []
    ts_need = 0
    for c in range(NCORES):
        spare = tb * P - len(bsplit[c])
        nbor = min(spare, len(ssplit[c]))
        rows_b.append((bsplit[c], ssplit[c][:nbor]))
        rows_s.append(ssplit[c][nbor:])
        borrow_n.append(nbor)
        ts_need = max(ts_need, len(ssplit[c]) - nbor)
    ts = -(-ts_need // P) if ts_need else 0
    return tb, ts, rows_b, rows_s


def make_in_maps(rep, adj_arc_in, adj_lab_in, adj_mask_in, adj_mask_loop, mask,
                 W_in, b_in, W_gate_in, b_gate_in, W_self, W_gate_self):
    rep_ = np.ascontiguousarray(np.asarray(rep, dtype=np.float32)).reshape(M, D)
    arc = np.asarray(adj_arc_in)
    lab = np.asarray(adj_lab_in)
    idx = arc[:, 0].astype(np.int64) * L + arc[:, 1].astype(np.int64)

    w_both = np.stack([np.asarray(W_in), np.asarray(W_self)]).astype(MM_NP)
    wg2 = np.concatenate([np.asarray(W_gate_in), np.asarray(W_gate_self)],
                         axis=1).astype(MM_NP)
    bg = np.asarray(b_gate_in, dtype=np.float32)[:, 0]
    b_in_f = np.asarray(b_in, dtype=np.float32)

    m2i = (np.asarray(adj_mask_in)[:, 0].astype(np.float32)) ** 2
    m2l = (np.asarray(adj_mask_loop)[:, 0].astype(np.float32)) ** 2
    mk = np.asarray(mask, dtype=np.float32).reshape(M)

    if DENSE:
        tb, ts, rows_b, rows_s = plan_rows(np.ones(M, np.float32),
                                           np.ones(M, np.float32))
    else:
        tb, ts, rows_b, rows_s = plan_rows(mk, np.asarray(adj_mask_in)[:, 0])

    in_maps, scatter = [], []
    for c in range(NCORES):
        bidx, borrowed = rows_b[c]
        nb, nbor = len(bidx), len(borrowed)
        rb = np.concatenate([bidx, borrowed]).astype(np.int64)
        npad_b = tb * P - len(rb)
        rb_full = np.concatenate([rb, np.zeros(npad_b, np.int64)])

        # both-tile X: src0 = gathered in-arc rows (zeroed for borrowed/pad),
        # src1 = self rows (zeroed for pad)
        g_rows = rep_[idx[rb_full]]
        g_rows[nb:] = 0.0
        s_rows = rep_[rb_full]
        if npad_b:
            s_rows[nb + nbor:] = 0.0
        xb = np.stack([g_rows, s_rows])                 # (2, tbP, D)
        v = xb.reshape(2, tb, P, KT, P)                 # [s, m, c, k, p]
        xtb_c = np.ascontiguousarray(v.transpose(1, 4, 0, 3, 2)).astype(MM_NP)

        m2i_r = m2i[rb_full]
        m2i_r[nb:] = 0.0
        mk_r = mk[rb_full]
        if npad_b:
            mk_r[nb + nbor:] = 0.0
        bg_r = bg[lab[rb_full]]
        ones = np.ones(tb * P, np.float32)
        zc = np.zeros(tb * P, np.float32)
        msk_c = np.ascontiguousarray(np.stack(
            [m2i_r.reshape(tb, P).T, zc.reshape(tb, P).T, zc.reshape(tb, P).T,
             ones.reshape(tb, P).T, mk_r.reshape(tb, P).T,
             bg_r.reshape(tb, P).T], axis=2)).astype(np.float32)

        im = {"xtb": xtb_c, "w": w_both, "wg": wg2, "msk": msk_c}

        if BIAS == "dve":
            br = b_in_f[lab[rb_full]]
            br[nb:] = 0.0
            im["brow"] = br.astype(MM_NP).reshape(tb, P, O)
        else:
            im["bau"] = b_in_f.astype(MM_NP)
            oh = (lab[rb_full][None, :] == np.arange(R)[:, None])
            oh = oh & (np.arange(tb * P)[None, :] < nb)
            im["oht"] = oh.astype(MM_NP)

        sidx = rows_s[c].astype(np.int64)
        if ts:
            npad_s = ts * P - len(sidx)
            rs_full = np.concatenate([sidx, np.zeros(npad_s, np.int64)])
            srows = rep_[rs_full]
            if npad_s:
                srows[len(sidx):] = 0.0
            vs = srows.reshape(ts, P, KT, P)            # [m, c, k, p]
            im["xts"] = np.ascontiguousarray(
                vs.transpose(0, 3, 2, 1)).astype(MM_NP)
            mks_r = mk[rs_full]
            if npad_s:
                mks_r[len(sidx):] = 0.0
            im["msks"] = np.ascontiguousarray(
                mks_r.reshape(ts, P).T[:, :, None]).astype(np.float32)

        in_maps.append(im)
        scatter.append((rb, sidx))
    return tb, ts, in_maps, scatter


def kernel(**inputs):
    import time
    tb, ts, in_maps, scatter = make_in_maps(**inputs)
    nc = _get_nc(tb, ts)
    last = None
    for attempt in range(3):
        try:
            res = run_bass_kernel_spmd(nc, in_maps, core_ids=list(range(NCORES)))
            full = np.zeros((M, O), np.float32)
            for c in range(NCORES):
                oc = np.asarray(res.results[c]["out"]).astype(np.float32)
                rb, sidx = scatter[c]
                full[rb] = oc[:len(rb)]
                if len(sidx):
                    full[sidx] = oc[tb * P:tb * P + len(sidx)]
            return full.reshape(BNK, L, O)
        except Exception as e:  # transient device/tunnel errors: back off and retry
            last = e
            time.sleep(20 * (attempt + 1))
    raise last
